# revision 7
# baseline (speedup 1.0000x reference)
"""Trainium2 Bass kernel for nn_AttentionLayer (dense transformer layer).

Sharding: data-parallel over 8 NeuronCores. Each core owns 512 query tokens
(cores 0-3 -> batch 0, cores 4-7 -> batch 1) and recomputes K/V for its full
batch (2048 tokens) locally, so no collectives are needed.

Layouts (per core):
  - x^T tiles come pre-transposed from the host (free layout prep).
  - Q^T, K^T are produced directly in [d_out, token] layout (weights as the
    stationary operand); V in [token, d_out] layout with an interleaved
    ones-column per head (65-wide blocks) so the PV matmul also produces the
    softmax denominators (row 64 of each PV psum).
  - Scores are computed transposed: s^T[k, q] = K_h^T(stationary) x Q_h^T.
    Softmax needs no max-subtraction (|s|/8 is small for this data) and no
    reduction ops: exp via ScalarE straight from PSUM, denominators via the
    V ones-row, normalization deferred to after PV via K=1 broadcast matmuls.
  - All matmuls run in float32r (full PE rate, ~1e-4 relative precision).
Attention runs in two k-halves of 1024 tokens so K^T/V halves fit SBUF.
Softmax sums and the normalized attention output bounce through DRAM scratch
so the attention pools can be freed before the out-proj/FFN phases.
"""

import numpy as np
import concourse.bacc as bacc
import concourse.mybir as mybir
import concourse.tile as tile
from concourse import bass_utils

F32 = mybir.dt.float32
F32R = mybir.dt.float32r
AF = mybir.ActivationFunctionType
ALU = mybir.AluOpType
AX = mybir.AxisListType

D = 1024
FFN_D = 4096
NH = 16          # heads
DH = 64          # head dim
B = 2
L = 2048
NCORES = 8
T = 512          # query tokens per core
KHALF = 1024     # k tokens per attention half
LN_EPS = 1e-5


def _layernorm(nc, scr, small, src_ap, dst_ap, g_t, b_t, eps_t):
    """LN over the free dim (D=1024) of a [128, 1024] slice."""
    mu = small.tile([128, 1], F32, tag="mu")
    nc.vector.tensor_reduce(mu[:], src_ap, AX.X, ALU.add)
    nc.vector.tensor_scalar_mul(mu[:], mu[:], 1.0 / D)
    cen = scr.tile([128, D], F32, tag="cen")
    nc.vector.tensor_scalar(cen[:], src_ap, mu[:], None, ALU.subtract)
    sq = scr.tile([128, D], F32, tag="sq")
    nc.vector.tensor_tensor(sq[:], cen[:], cen[:], ALU.mult)
    ss = small.tile([128, 1], F32, tag="ss")
    nc.vector.tensor_reduce(ss[:], sq[:], AX.X, ALU.add)
    std = small.tile([128, 1], F32, tag="std")
    nc.scalar.activation(std[:], ss[:], AF.Sqrt, bias=eps_t[:], scale=1.0 / D)
    rstd = small.tile([128, 1], F32, tag="rstd")
    nc.vector.reciprocal(rstd[:], std[:])
    nc.vector.scalar_tensor_tensor(dst_ap, cen[:], rstd[:], g_t[:], ALU.mult, ALU.mult)
    nc.vector.tensor_tensor(dst_ap, dst_ap, b_t[:], ALU.add)


def build_program():
    nc = bacc.Bacc("TRN2", target_bir_lowering=False, debug=False)

    xTb = nc.dram_tensor("xTb", [D, L], F32, kind="ExternalInput")
    xTl = nc.dram_tensor("xTl", [D, T], F32, kind="ExternalInput")
    xl = nc.dram_tensor("xl", [T, D], F32, kind="ExternalInput")
    wq = nc.dram_tensor("wq", [D, D], F32, kind="ExternalInput")
    wk = nc.dram_tensor("wk", [D, D], F32, kind="ExternalInput")
    wv = nc.dram_tensor("wv", [D, D], F32, kind="ExternalInput")
    wo = nc.dram_tensor("wo", [D, D], F32, kind="ExternalInput")
    w1 = nc.dram_tensor("w1", [D, FFN_D], F32, kind="ExternalInput")
    w2 = nc.dram_tensor("w2", [FFN_D, D], F32, kind="ExternalInput")
    bqr = nc.dram_tensor("bqr", [8, 128, 1], F32, kind="ExternalInput")
    bkr = nc.dram_tensor("bkr", [8, 128, 1], F32, kind="ExternalInput")
    bf1r = nc.dram_tensor("bf1r", [32, 128, 1], F32, kind="ExternalInput")
    bvb = nc.dram_tensor("bvb", [128, D], F32, kind="ExternalInput")
    bob = nc.dram_tensor("bob", [128, D], F32, kind="ExternalInput")
    bf2b = nc.dram_tensor("bf2b", [128, D], F32, kind="ExternalInput")
    g1b = nc.dram_tensor("g1b", [128, D], F32, kind="ExternalInput")
    b1b = nc.dram_tensor("b1b", [128, D], F32, kind="ExternalInput")
    g2b = nc.dram_tensor("g2b", [128, D], F32, kind="ExternalInput")
    b2b = nc.dram_tensor("b2b", [128, D], F32, kind="ExternalInput")
    ident = nc.dram_tensor("ident", [128, 128], F32, kind="ExternalInput")
    out = nc.dram_tensor("out", [T, D], F32, kind="ExternalOutput")

    with tile.TileContext(nc) as tc:
        with (
            tc.tile_pool(name="consts", bufs=1) as cpool,
            tc.tile_pool(name="dscr", bufs=1, space="DRAM") as dscr,
        ):
            ident_t = cpool.tile([128, 128], F32, tag="ident")
            nc.sync.dma_start(ident_t[:], ident[:, :])
            ones_st = cpool.tile([128, 128], F32, tag="ones_st")
            nc.vector.memset(ones_st[:], 1.0)
            eps_t = cpool.tile([128, 1], F32, tag="eps")
            nc.vector.memset(eps_t[:], LN_EPS)

            sums_d = dscr.tile([NH, T], F32, tag="sums_d")      # softmax denoms
            recips_d = dscr.tile([NH, T], F32, tag="recips_d")  # 1/denoms
            aoT_d = dscr.tile([128, 8, T], F32R, tag="aoT_d")   # attnout^T

            # =================== attention block ===================
            with (
                tc.tile_pool(name="accp", bufs=1) as accp,
                tc.tile_pool(name="qtp", bufs=1) as qtp,
            ):
                acc = accp.tile([128, 8, T], F32, tag="acc")
                qt = qtp.tile([128, 8, T], F32R, tag="qt")

                # ---- Q projection: qt[d, 8, q] = wq^T @ x_loc^T ----
                with (
                    tc.tile_pool(name="qx", bufs=8) as qx,
                    tc.tile_pool(name="qw", bufs=2) as qw,
                    tc.tile_pool(name="qb", bufs=2) as qb,
                    tc.tile_pool(name="psQ", bufs=2, space="PSUM") as psQ,
                ):
                    xts = []
                    for di in range(8):
                        xt_t = qx.tile([128, T], F32R, tag="qx")
                        nc.sync.dma_start(
                            xt_t[:], xTl[di * 128:(di + 1) * 128, :].bitcast(F32R))
                        xts.append(xt_t)
                    for do in range(8):
                        wqt = qw.tile([128, 8, 128], F32R, tag="qw")
                        nc.sync.dma_start(
                            wqt[:],
                            wq[:, do * 128:(do + 1) * 128]
                            .rearrange("(a p) n -> p a n", p=128).bitcast(F32R))
                        ps = psQ.tile([128, T], F32, tag="psQ")
                        for di in range(8):
                            nc.tensor.matmul(ps[:], wqt[:, di, :], xts[di][:],
                                             start=(di == 0), stop=(di == 7))
                        bq_t = qb.tile([128, 1], F32, tag="qb")
                        nc.sync.dma_start(bq_t[:], bqr[do])
                        nc.vector.tensor_scalar_add(qt[:, do, :], ps[:], bq_t[:])

                # ---- two k-halves: project K^T/V then attend ----
                for half in range(2):
                    with tc.tile_pool(name="kvt", bufs=1) as kvt:
                        kt = kvt.tile([128, 8, KHALF], F32R, tag="kt")
                        vt = kvt.tile([128, 8, 1040], F32R, tag="vt")

                        with (
                            tc.tile_pool(name="px", bufs=8) as px,
                            tc.tile_pool(name="pw", bufs=2) as pw,
                            tc.tile_pool(name="pwv", bufs=8) as pwv,
                            tc.tile_pool(name="pb", bufs=2) as pb,
                            tc.tile_pool(name="pbv", bufs=1) as pbv,
                            tc.tile_pool(name="psP", bufs=4, space="PSUM") as psP,
                        ):
                            bvb_t = pbv.tile([128, D], F32, tag="bvb")
                            nc.sync.dma_start(bvb_t[:], bvb[:, :])
                            for tc2 in range(2):
                                tok0 = half * KHALF + tc2 * 512
                                xts = []
                                for di in range(8):
                                    xt_t = px.tile([128, 512], F32R, tag="px")
                                    nc.sync.dma_start(
                                        xt_t[:],
                                        xTb[di * 128:(di + 1) * 128,
                                            tok0:tok0 + 512].bitcast(F32R))
                                    xts.append(xt_t)
                                # K^T for these 512 tokens
                                for do in range(8):
                                    wkt = pw.tile([128, 8, 128], F32R, tag="wk")
                                    nc.sync.dma_start(
                                        wkt[:],
                                        wk[:, do * 128:(do + 1) * 128]
                                        .rearrange("(a p) n -> p a n", p=128)
                                        .bitcast(F32R))
                                    ps = psP.tile([128, 512], F32, tag="psP")
                                    for di in range(8):
                                        nc.tensor.matmul(ps[:], wkt[:, di, :],
                                                         xts[di][:],
                                                         start=(di == 0),
                                                         stop=(di == 7))
                                    bk_t = pb.tile([128, 1], F32, tag="pb")
                                    nc.sync.dma_start(bk_t[:], bkr[do])
                                    nc.vector.tensor_scalar_add(
                                        kt[:, do, tc2 * 512:(tc2 + 1) * 512],
                                        ps[:], bk_t[:])
                                # V for these 512 tokens (65-wide head blocks)
                                for n in range(2):
                                    wvts = []
                                    for di in range(8):
                                        wvt = pwv.tile([128, 512], F32R, tag="wv")
                                        nc.sync.dma_start(
                                            wvt[:],
                                            wv[di * 128:(di + 1) * 128,
                                               n * 512:(n + 1) * 512].bitcast(F32R))
                                        wvts.append(wvt)
                                    for tt in range(4):
                                        kcl = tc2 * 4 + tt
                                        vrow = vt[:, kcl, :].rearrange(
                                            "p (h c) -> p h c", c=65)
                                        if n == 0:
                                            nc.vector.tensor_copy(
                                                vrow[:, :, 64:65],
                                                ones_st[:, 0:16].rearrange(
                                                    "p (h c) -> p h c", c=1))
                                        ps = psP.tile([128, 512], F32, tag="psP")
                                        for di in range(8):
                                            nc.tensor.matmul(
                                                ps[:],
                                                xts[di][:, tt * 128:(tt + 1) * 128],
                                                wvts[di][:],
                                                start=(di == 0), stop=(di == 7))
                                        dest = vrow[:, n * 8:(n + 1) * 8, 0:64]
                                        src = ps[:].rearrange(
                                            "p (h c) -> p h c", c=64)
                                        bias = bvb_t[:, n * 512:(n + 1) * 512] \
                                            .rearrange("p (h c) -> p h c", c=64)
                                        nc.vector.scalar_tensor_tensor(
                                            dest, src, 1.0, bias,
                                            ALU.mult, ALU.add)

                        # ---- attention over this half ----
                        with (
                            tc.tile_pool(name="expp", bufs=3) as expp,
                            tc.tile_pool(name="sstage", bufs=2) as sstage,
                            tc.tile_pool(name="psS", bufs=2, space="PSUM") as psS,
                            tc.tile_pool(name="psV", bufs=4, space="PSUM") as psV,
                        ):
                            for hp in range(8):
                                h0, h1 = 2 * hp, 2 * hp + 1
                                pv0 = psV.tile([128, T], F32, tag="psV")
                                pv1 = psV.tile([128, T], F32, tag="psV")
                                for kcl in range(8):
                                    ss = psS.tile([128, 1024], F32, tag="psS")
                                    nc.tensor.matmul(
                                        ss[:, 0:512],
                                        kt[0:64, hp, kcl * 128:(kcl + 1) * 128],
                                        qt[0:64, hp, :],
                                        start=True, stop=True,
                                        tile_position=(0, 0))
                                    nc.tensor.matmul(
                                        ss[:, 512:1024],
                                        kt[64:128, hp, kcl * 128:(kcl + 1) * 128],
                                        qt[64:128, hp, :],
                                        start=True, stop=True,
                                        tile_position=(64, 0))
                                    ex = expp.tile([128, 1024], F32R, tag="exp")
                                    nc.scalar.activation(ex[:], ss[:], AF.Exp,
                                                         scale=0.125)
                                    vrow = vt[:, kcl, :].rearrange(
                                        "p (h c) -> p h c", c=65)
                                    nc.tensor.matmul(pv0[0:65, :], vrow[:, h0, :],
                                                     ex[:, 0:512],
                                                     start=(kcl == 0),
                                                     stop=(kcl == 7))
                                    nc.tensor.matmul(pv1[0:65, :], vrow[:, h1, :],
                                                     ex[:, 512:1024],
                                                     start=(kcl == 0),
                                                     stop=(kcl == 7))
                                for h, pv in ((h0, pv0), (h1, pv1)):
                                    a_slice = acc[(h % 2) * 64:(h % 2) * 64 + 64,
                                                  h // 2, :]
                                    stg = sstage.tile([1, T], F32, tag="stg")
                                    nc.vector.tensor_copy(stg[:], pv[64:65, :])
                                    if half == 0:
                                        nc.vector.tensor_copy(a_slice, pv[0:64, :])
                                        nc.sync.dma_start(sums_d[h:h + 1, :],
                                                          stg[:])
                                    else:
                                        nc.vector.tensor_tensor(
                                            a_slice, a_slice, pv[0:64, :], ALU.add)
                                        nc.gpsimd.dma_start(
                                            sums_d[h:h + 1, :], stg[:],
                                            accum_op=ALU.add)

                # ---- normalize: aoT = acc / sums, staged out to DRAM ----
                with (
                    tc.tile_pool(name="nrm", bufs=4) as nrm,
                    tc.tile_pool(name="sload", bufs=4) as sload,
                ):
                    for h in range(NH):
                        sl = sload.tile([1, T], F32, tag="sl")
                        nc.sync.dma_start(sl[:], sums_d[h:h + 1, :])
                        rc = sload.tile([1, T], F32, tag="rc")
                        nc.vector.reciprocal(rc[:], sl[:])
                        nc.sync.dma_start(recips_d[h:h + 1, :], rc[:])
                    for hp in range(8):
                        h0, h1 = 2 * hp, 2 * hp + 1
                        bc = nrm.tile([128, T], F32, tag="bc")
                        nc.sync.dma_start(
                            bc[0:64, :],
                            recips_d[h0:h0 + 1, :].to_broadcast([64, T]))
                        nc.sync.dma_start(
                            bc[64:128, :],
                            recips_d[h1:h1 + 1, :].to_broadcast([64, T]))
                        ao = nrm.tile([128, T], F32R, tag="ao")
                        nc.vector.tensor_tensor(ao[0:64, :], acc[0:64, hp, :],
                                                bc[0:64, :], ALU.mult)
                        nc.vector.tensor_tensor(ao[64:128, :], acc[64:128, hp, :],
                                                bc[64:128, :], ALU.mult)
                        nc.sync.dma_start(aoT_d[:, hp, :], ao[:])

            # ============ out-projection + residual + LN1 ============
            with tc.tile_pool(name="ln1p", bufs=1) as ln1p:
                ln1 = ln1p.tile([128, 4, D], F32, tag="ln1")
                with (
                    tc.tile_pool(name="aosb", bufs=1) as aosb,
                    tc.tile_pool(name="xres", bufs=1) as xres,
                    tc.tile_pool(name="res1p", bufs=1) as res1p,
                    tc.tile_pool(name="wop", bufs=3) as wop,
                    tc.tile_pool(name="obias", bufs=1) as obias,
                    tc.tile_pool(name="scr", bufs=2) as scr,
                    tc.tile_pool(name="small", bufs=4) as small,
                    tc.tile_pool(name="psO", bufs=8, space="PSUM") as psO,
                ):
                    aoT = aosb.tile([128, 8, T], F32R, tag="aoT")
                    nc.sync.dma_start(aoT[:], aoT_d[:])
                    x_loc = xres.tile([128, 4, D], F32, tag="xloc")
                    nc.sync.dma_start(x_loc[:],
                                      xl.rearrange("(a p) n -> p a n", p=128))
                    res1 = res1p.tile([128, 4, D], F32, tag="res1")
                    bob_t = obias.tile([128, D], F32, tag="bob")
                    nc.sync.dma_start(bob_t[:], bob[:, :])
                    g1_t = obias.tile([128, D], F32, tag="g1")
                    nc.sync.dma_start(g1_t[:], g1b[:, :])
                    b1_t = obias.tile([128, D], F32, tag="b1")
                    nc.sync.dma_start(b1_t[:], b1b[:, :])
                    for n in range(2):
                        pss = [psO.tile([128, 512], F32, tag="psO",
                                        name=f"psO_{n}_{i}") for i in range(4)]
                        for c in range(8):
                            wot = wop.tile([128, 512], F32R, tag="wo")
                            nc.sync.dma_start(
                                wot[:],
                                wo[c * 128:(c + 1) * 128,
                                   n * 512:(n + 1) * 512].bitcast(F32R))
                            for qt4 in range(4):
                                nc.tensor.matmul(
                                    pss[qt4][:],
                                    aoT[:, c, qt4 * 128:(qt4 + 1) * 128],
                                    wot[:], start=(c == 0), stop=(c == 7))
                        for qt4 in range(4):
                            sl = (slice(None), qt4, slice(n * 512, (n + 1) * 512))
                            nc.vector.scalar_tensor_tensor(
                                res1[sl], pss[qt4][:], 1.0,
                                x_loc[sl], ALU.mult, ALU.add)
                            nc.vector.tensor_tensor(
                                res1[sl], res1[sl],
                                bob_t[:, n * 512:(n + 1) * 512], ALU.add)
                    for qt4 in range(4):
                        _layernorm(nc, scr, small, res1[:, qt4, :],
                                   ln1[:, qt4, :], g1_t, b1_t, eps_t)

                # =================== FFN ===================
                with (
                    tc.tile_pool(name="ln1tp", bufs=1) as ln1tp,
                    tc.tile_pool(name="midp", bufs=1) as midp,
                    tc.tile_pool(name="w1p", bufs=2) as w1p,
                    tc.tile_pool(name="w2p", bufs=3) as w2p,
                    tc.tile_pool(name="fbias", bufs=2) as fbias,
                    tc.tile_pool(name="fconst", bufs=1) as fconst,
                    tc.tile_pool(name="res2p", bufs=1) as res2p,
                    tc.tile_pool(name="outp", bufs=2) as outp,
                    tc.tile_pool(name="scr2", bufs=2) as scr2,
                    tc.tile_pool(name="small2", bufs=4) as small2,
                    tc.tile_pool(name="psT", bufs=2, space="PSUM") as psT,
                    tc.tile_pool(name="psF", bufs=2, space="PSUM") as psF,
                    tc.tile_pool(name="psF2", bufs=4, space="PSUM") as psF2,
                ):
                    ln1T = ln1tp.tile([128, 8, T], F32R, tag="ln1T")
                    for qt4 in range(4):
                        for dc in range(8):
                            pst = psT.tile([128, 128], F32, tag="psT")
                            nc.tensor.transpose(
                                pst[:], ln1[:, qt4, dc * 128:(dc + 1) * 128],
                                ident_t[:])
                            nc.vector.tensor_copy(
                                ln1T[:, dc, qt4 * 128:(qt4 + 1) * 128], pst[:])
                    midT = midp.tile([128, 32, T], F32R, tag="midT")
                    for ft in range(32):
                        w1t = w1p.tile([128, 8, 128], F32R, tag="w1")
                        nc.sync.dma_start(
                            w1t[:],
                            w1[:, ft * 128:(ft + 1) * 128]
                            .rearrange("(a p) n -> p a n", p=128).bitcast(F32R))
                        ps = psF.tile([128, T], F32, tag="psF")
                        for di in range(8):
                            nc.tensor.matmul(ps[:], w1t[:, di, :], ln1T[:, di, :],
                                             start=(di == 0), stop=(di == 7))
                        bf1_t = fbias.tile([128, 1], F32, tag="bf1")
                        nc.sync.dma_start(bf1_t[:], bf1r[ft])
                        nc.scalar.activation(midT[:, ft, :], ps[:], AF.Gelu,
                                             bias=bf1_t[:])
                    bf2_t = fconst.tile([128, D], F32, tag="bf2")
                    nc.sync.dma_start(bf2_t[:], bf2b[:, :])
                    g2_t = fconst.tile([128, D], F32, tag="g2")
                    nc.sync.dma_start(g2_t[:], g2b[:, :])
                    b2_t = fconst.tile([128, D], F32, tag="b2")
                    nc.sync.dma_start(b2_t[:], b2b[:, :])
                    res2 = res2p.tile([128, 4, D], F32, tag="res2")
                    for n in range(2):
                        pss = [psF2.tile([128, 512], F32, tag="psF2",
                                         name=f"psF2_{n}_{i}") for i in range(4)]
                        for fc in range(32):
                            w2t = w2p.tile([128, 512], F32R, tag="w2")
                            nc.sync.dma_start(
                                w2t[:],
                                w2[fc * 128:(fc + 1) * 128,
                                   n * 512:(n + 1) * 512].bitcast(F32R))
                            for qt4 in range(4):
                                nc.tensor.matmul(
                                    pss[qt4][:],
                                    midT[:, fc, qt4 * 128:(qt4 + 1) * 128],
                                    w2t[:], start=(fc == 0), stop=(fc == 31))
                        for qt4 in range(4):
                            sl = (slice(None), qt4, slice(n * 512, (n + 1) * 512))
                            nc.vector.scalar_tensor_tensor(
                                res2[sl], pss[qt4][:], 1.0,
                                ln1[sl], ALU.mult, ALU.add)
                            nc.vector.tensor_tensor(
                                res2[sl], res2[sl],
                                bf2_t[:, n * 512:(n + 1) * 512], ALU.add)
                    out_r = out.rearrange("(a p) n -> p a n", p=128)
                    for qt4 in range(4):
                        o_t = outp.tile([128, D], F32, tag="out")
                        _layernorm(nc, scr2, small2, res2[:, qt4, :],
                                   o_t[:], g2_t, b2_t, eps_t)
                        nc.sync.dma_start(out_r[:, qt4, :], o_t[:])

    nc.compile()
    return nc


_cached_nc = None


def _get_nc():
    global _cached_nc
    if _cached_nc is None:
        _cached_nc = build_program()
    return _cached_nc


def make_in_maps(x, wq, bq, wk, bk, wv, bv, wo, bo, g1, b1, g2, b2, w1, bf1, w2, bf2):
    x = np.ascontiguousarray(np.asarray(x, np.float32).reshape(B * L, D))
    xT = np.ascontiguousarray(x.T)                       # [D, B*L]
    common = {
        "wq": np.ascontiguousarray(wq, np.float32),
        "wk": np.ascontiguousarray(wk, np.float32),
        "wv": np.ascontiguousarray(wv, np.float32),
        "wo": np.ascontiguousarray(wo, np.float32),
        "w1": np.ascontiguousarray(w1, np.float32),
        "w2": np.ascontiguousarray(w2, np.float32),
        "bqr": np.asarray(bq, np.float32).reshape(8, 128, 1),
        "bkr": np.asarray(bk, np.float32).reshape(8, 128, 1),
        "bf1r": np.asarray(bf1, np.float32).reshape(32, 128, 1),
        "bvb": np.tile(np.asarray(bv, np.float32), (128, 1)),
        "bob": np.tile(np.asarray(bo, np.float32), (128, 1)),
        "bf2b": np.tile(np.asarray(bf2, np.float32), (128, 1)),
        "g1b": np.tile(np.asarray(g1, np.float32), (128, 1)),
        "b1b": np.tile(np.asarray(b1, np.float32), (128, 1)),
        "g2b": np.tile(np.asarray(g2, np.float32), (128, 1)),
        "b2b": np.tile(np.asarray(b2, np.float32), (128, 1)),
        "ident": np.eye(128, dtype=np.float32),
    }
    in_maps = []
    for c in range(NCORES):
        b = c // 4
        q0 = c * T
        m = dict(common)
        m["xTb"] = np.ascontiguousarray(xT[:, b * L:(b + 1) * L])
        m["xTl"] = np.ascontiguousarray(xT[:, q0:q0 + T])
        m["xl"] = np.ascontiguousarray(x[q0:q0 + T, :])
        in_maps.append(m)
    return in_maps


def kernel(**inputs):
    nc = _get_nc()
    in_maps = make_in_maps(**inputs)
    res = bass_utils.run_bass_kernel_spmd(nc, in_maps, core_ids=list(range(NCORES)))
    pieces = [res.results[c]["out"] for c in range(NCORES)]
    return np.concatenate(pieces, axis=0).reshape(B, L, D)


if __name__ == "__main__":
    build_program()
    print("BUILD OK")
